# revision 7
# baseline (speedup 1.0000x reference)
"""T5-style MultiHeadAttention (relative position bias) on 8 Trainium2 cores.

Sharding: core c = (b, g) with b = c // 4 (batch), g = c % 4 (head group of 4
heads).  Each core computes q/k/v projections for its 4 heads, attention with
the relative-position bias, and a partial output projection (rows of Wo for
its heads).  Host sums the 4 partials per batch element.

Key layout choices (per core):
  - x is passed transposed: xT [1024, 2048] so projections contract over
    partitions directly.
  - Q_t, K_t stored as [d, seq] (d on partitions); scores computed
    *transposed* as S_t[k, q] = K_t^T-stationary matmul, so that exp(S_t) can
    be used directly as the stationary operand of the attn@V matmul (which
    contracts over k = partitions).
  - Softmax denominator Z[q] = sum_k exp(S_t[k, q]) falls out of the attn@V
    matmul for free via a ones-column appended to V (M=65 per head).
  - No max-subtraction: scores are O(50) at most, exp stays finite in fp32
    and bf16.
  - Relative-position bias applied multiplicatively after exp:
    exp(s + bias) = exp(s) * exp(bias).  bias[k, q] = v_h[k - q + 2047] is
    Toeplitz, so exp(bias) tiles are overlapping windows of a single
    [128, 3968] sliding table per head, precomputed on host:
      T_h[p, i] = exp(v_h[p + 3967 - i])
    and the tile for (k0 = kc*128, q0 = qb*512) is T_h[:, base:base+512] with
    base = 1920 - k0 + q0.
  - Matmuls run in float32r (full PE rate at N>=256); the attention
    probabilities / V use bf16 (configurable) for the 2x DVE multiply mode.
"""

import numpy as np
from contextlib import ExitStack

import concourse.bass as bass
import concourse.tile as tile
from concourse import bacc, mybir
from concourse.bass_utils import run_bass_kernel_spmd

# ---------------------------------------------------------------- constants
B, S, D_MODEL, N_HEADS, D_KV = 2, 2048, 1024, 16, 64
NUM_BUCKETS, MAX_DIST = 32, 128
N_CORES = 8
HPC = N_HEADS // (N_CORES // B)  # heads per core = 4
DH = HPC * D_KV                  # 256 d-cols per core
TBL = 3968                       # exp-bias sliding table width
QB = 512                         # q block (free dim of score tiles)
KC = 128                         # k chunk (partition dim of score tiles)

F32 = mybir.dt.float32
F32R = mybir.dt.float32r
BF16 = mybir.dt.bfloat16
AF = mybir.ActivationFunctionType

# attention-probability dtype: BF16 (fast DVE 2x) or F32 (accurate, 1x DVE)
ATT_DT = BF16

_cache = {}


# ------------------------------------------------------------- host helpers
def _rel_bucket(d):
    """Bucket of relative position d = k - q (bidirectional T5), numpy fp32
    mirror of the jax reference."""
    nb = NUM_BUCKETS // 2
    n = -d
    ret = (n < 0).astype(np.int32) * nb
    n = np.abs(n)
    max_exact = nb // 2
    is_small = n < max_exact
    nf = np.maximum(n, 1).astype(np.float32)
    val = (
        np.log(nf / np.float32(max_exact))
        / np.float32(np.log(MAX_DIST / max_exact))
        * np.float32(nb - max_exact)
    ).astype(np.int32) + max_exact
    val = np.minimum(val, nb - 1)
    return ret + np.where(is_small, n, val)


def _expbias_tables(rel_emb):
    """[N_HEADS, 128, TBL] exp-bias sliding tables (float32)."""
    d = np.arange(-(S - 1), S)  # k - q in [-2047, 2047]
    buck = _rel_bucket(d)  # [4095]
    vals = rel_emb[buck, :].astype(np.float32)  # [4095, H]
    idx = np.arange(KC)[:, None] + (TBL - 1) - np.arange(TBL)[None, :]
    t = np.exp(vals[idx, :])  # [128, TBL, H]
    return np.ascontiguousarray(np.transpose(t, (2, 0, 1)))


# ------------------------------------------------------------- kernel body
def mha_body(tc, outs, ins, ckpt=None):
    nc = tc.nc
    ctx = ExitStack()
    xt_d = ins["xt"]        # [1024, 2048] f32
    wq_d = ins["wq"]        # [1024, 256]
    wk_d = ins["wk"]        # [1024, 256]
    wv_d = ins["wv"]        # [1024, 256]
    wo_d = ins["wo"]        # [256, 1024]
    eb_d = ins["expb"]      # [HPC, 128, TBL] ATT_DT
    out_d = outs["out"]     # [2048, 1024] f32

    att_np = ATT_DT
    DKN = D_MODEL // 128    # 8 contraction chunks
    NQ = S // QB            # 4 q blocks
    NK = S // KC            # 16 k chunks

    def r(ap):  # operands already float32r
        return ap

    with ctx:
        const = ctx.enter_context(tc.tile_pool(name="const", bufs=1))

        # ---- persistent SBUF tensors
        qt = [const.tile([128, S], F32R, tag=f"qt{i}", name=f"qt{i}") for i in range(2)]
        kt = [const.tile([128, S], F32R, tag=f"kt{i}", name=f"kt{i}") for i in range(2)]
        # V with a ones column per head: [k, 4*65]; bf16 (AV stationary)
        vsb = [const.tile([128, HPC * 65], att_np, tag=f"v{i}", name=f"v{i}") for i in range(NK)]
        usb = [const.tile([64, S], F32R, tag=f"u{i}", name=f"u{i}") for i in range(HPC)]
        wo = [const.tile([64, D_MODEL], F32R, tag=f"wo{i}", name=f"wo{i}") for i in range(HPC)]

        for h in range(HPC):
            nc.sync.dma_start(out=wo[h], in_=wo_d[h * 64:(h + 1) * 64, :])

        # ================= phase 1: projections =================
        with tc.tile_pool(name="wqkv", bufs=1) as wpool, \
             tc.tile_pool(name="qkps", bufs=2, space="PSUM") as qkps, \
             tc.tile_pool(name="vps", bufs=4, space="PSUM") as vps:
            xt = [wpool.tile([128, S], F32R, tag=f"xt{i}", name=f"xt{i}") for i in range(DKN)]
            for i in range(DKN):
                nc.sync.dma_start(out=xt[i], in_=xt_d[i * 128:(i + 1) * 128, :])
            wq = [wpool.tile([128, DH], F32R, tag=f"wq{i}", name=f"wq{i}") for i in range(DKN)]
            wk = [wpool.tile([128, DH], F32R, tag=f"wk{i}", name=f"wk{i}") for i in range(DKN)]
            wv = [wpool.tile([128, DH], F32R, tag=f"wv{i}", name=f"wv{i}") for i in range(DKN)]
            for i in range(DKN):
                nc.sync.dma_start(out=wq[i], in_=wq_d[i * 128:(i + 1) * 128, :])
                nc.sync.dma_start(out=wk[i], in_=wk_d[i * 128:(i + 1) * 128, :])
                nc.sync.dma_start(out=wv[i], in_=wv_d[i * 128:(i + 1) * 128, :])

            # Q_t/K_t: [256, 2048] as 2 tiles of [128, 2048]
            for m in range(2):
                for qb in range(NQ):
                    pq = qkps.tile([128, QB], F32, tag="pq")
                    pk = qkps.tile([128, QB], F32, tag="pk")
                    for dk in range(DKN):
                        nc.tensor.matmul(
                            pq, r(wq[dk][:, m * 128:(m + 1) * 128]),
                            r(xt[dk][:, qb * QB:(qb + 1) * QB]),
                            start=(dk == 0), stop=(dk == DKN - 1))
                    for dk in range(DKN):
                        nc.tensor.matmul(
                            pk, r(wk[dk][:, m * 128:(m + 1) * 128]),
                            r(xt[dk][:, qb * QB:(qb + 1) * QB]),
                            start=(dk == 0), stop=(dk == DKN - 1))
                    nc.scalar.copy(out=qt[m][:, qb * QB:(qb + 1) * QB], in_=pq)
                    nc.scalar.copy(out=kt[m][:, qb * QB:(qb + 1) * QB], in_=pk)

            # V: [2048, 256] -> per k-chunk [128, 4*65] bf16 (+ ones cols)
            for kc in range(NK):
                pv = vps.tile([128, DH], F32, tag="pv")
                for dk in range(DKN):
                    nc.tensor.matmul(
                        pv, r(xt[dk][:, kc * 128:(kc + 1) * 128]), r(wv[dk]),
                        start=(dk == 0), stop=(dk == DKN - 1))
                v3 = vsb[kc].rearrange("p (h c) -> p h c", h=HPC)
                nc.scalar.copy(
                    out=v3[:, :, 0:64],
                    in_=pv.rearrange("p (h c) -> p h c", h=HPC))
                nc.vector.memset(v3[:, :, 64:65], 1.0)

        # ================= phase 2+3: attention =================
        with tc.tile_pool(name="expb", bufs=2) as ebp, \
             tc.tile_pool(name="es", bufs=4) as esp, \
             tc.tile_pool(name="esb", bufs=4) as esbp, \
             tc.tile_pool(name="rz", bufs=2) as rzp, \
             tc.tile_pool(name="outsb", bufs=3) as outp, \
             tc.tile_pool(name="sps", bufs=4, space="PSUM") as sps, \
             tc.tile_pool(name="ups", bufs=2, space="PSUM") as ups, \
             tc.tile_pool(name="ops", bufs=2, space="PSUM") as ops:

            for h in range(HPC):
                eb = ebp.tile([128, TBL], att_np, tag="eb")
                nc.sync.dma_start(out=eb, in_=eb_d[h])
                hp, hh = h // 2, h % 2
                prow = slice(hh * 64, hh * 64 + 64)
                for qb in range(NQ):
                    pu = ups.tile([65, QB], F32, tag="pu")
                    for kc in range(NK):
                        ps = sps.tile([128, QB], F32, tag="ps")
                        nc.tensor.matmul(
                            ps,
                            r(kt[hp][prow, kc * 128:(kc + 1) * 128]),
                            r(qt[hp][prow, qb * QB:(qb + 1) * QB]),
                            start=True, stop=True)
                        es = esp.tile([128, QB], att_np, tag="es")
                        nc.scalar.activation(out=es, in_=ps, func=AF.Exp)
                        esb = esbp.tile([128, QB], att_np, tag="esb")
                        base = (TBL - S) - kc * 128 + qb * QB
                        nc.vector.tensor_mul(esb, es, eb[:, base:base + QB])
                        nc.tensor.matmul(
                            pu, vsb[kc][:, h * 65:(h + 1) * 65], esb,
                            start=(kc == 0), stop=(kc == NK - 1))
                    # normalize: U[d, q] / Z[q],  Z = row 64 of pu
                    rz = rzp.tile([1, QB], F32, tag="rz")
                    nc.vector.reciprocal(out=rz, in_=pu[64:65, :])
                    rzb = rzp.tile([64, QB], F32, tag="rzb")
                    nc.gpsimd.partition_broadcast(rzb, rz, channels=64)
                    nc.vector.tensor_mul(
                        usb[h][:, qb * QB:(qb + 1) * QB], pu[0:64, :], rzb)

            # ================= phase 4: output projection =================
            for qc in range(S // 128):
                ob = outp.tile([128, D_MODEL], F32, tag="ob")
                for e in range(2):
                    po = ops.tile([128, 512], F32, tag="po")
                    for h in range(HPC):
                        nc.tensor.matmul(
                            po,
                            r(usb[h][:, qc * 128:(qc + 1) * 128]),
                            r(wo[h][:, e * 512:(e + 1) * 512]),
                            start=(h == 0), stop=(h == HPC - 1))
                    nc.vector.tensor_copy(out=ob[:, e * 512:(e + 1) * 512], in_=po)
                nc.sync.dma_start(out=out_d[qc * 128:(qc + 1) * 128, :], in_=ob)


# ------------------------------------------------------------- build + run
def _build():
    if "nc" in _cache:
        return _cache["nc"]
    nc = bacc.Bacc("TRN2", target_bir_lowering=False, debug=False)
    att_np_dt = mybir.dt.np(ATT_DT)
    ins = {
        "xt": nc.dram_tensor("xt", [D_MODEL, S], F32R, kind="ExternalInput").ap(),
        "wq": nc.dram_tensor("wq", [D_MODEL, DH], F32R, kind="ExternalInput").ap(),
        "wk": nc.dram_tensor("wk", [D_MODEL, DH], F32R, kind="ExternalInput").ap(),
        "wv": nc.dram_tensor("wv", [D_MODEL, DH], F32R, kind="ExternalInput").ap(),
        "wo": nc.dram_tensor("wo", [DH, D_MODEL], F32R, kind="ExternalInput").ap(),
        "expb": nc.dram_tensor("expb", [HPC, KC, TBL], ATT_DT,
                               kind="ExternalInput").ap(),
    }
    outs = {
        "out": nc.dram_tensor("out", [S, D_MODEL], F32, kind="ExternalOutput").ap(),
    }
    with tile.TileContext(nc) as tc:
        mha_body(tc, outs, ins)
    nc.compile()
    _cache["nc"] = nc
    return nc


TRACE = False
LAST = {}


def kernel(inputs, Wq, Wk, Wv, Wo, rel_emb):
    inputs = np.asarray(inputs, dtype=np.float32)
    Wq = np.asarray(Wq, dtype=np.float32)
    Wk = np.asarray(Wk, dtype=np.float32)
    Wv = np.asarray(Wv, dtype=np.float32)
    Wo = np.asarray(Wo, dtype=np.float32)
    rel_emb = np.asarray(rel_emb, dtype=np.float32)

    nc = _build()
    att_np_dt = mybir.dt.np(ATT_DT)

    ebt = _expbias_tables(rel_emb)  # [16, 128, TBL] f32
    in_maps = []
    for c in range(N_CORES):
        b, g = c // (N_CORES // B), c % (N_CORES // B)
        hs = slice(g * DH, (g + 1) * DH)
        in_maps.append({
            "xt": np.ascontiguousarray(inputs[b].T),
            "wq": np.ascontiguousarray(Wq[:, hs]),
            "wk": np.ascontiguousarray(Wk[:, hs]),
            "wv": np.ascontiguousarray(Wv[:, hs]),
            "wo": np.ascontiguousarray(Wo[hs, :]),
            "expb": np.ascontiguousarray(
                ebt[g * HPC:(g + 1) * HPC]).astype(att_np_dt),
        })

    res = run_bass_kernel_spmd(
        nc, in_maps, core_ids=list(range(N_CORES)), trace=TRACE)
    LAST["res"] = res

    out = np.zeros((B, S, D_MODEL), dtype=np.float64)
    for c in range(N_CORES):
        b = c // (N_CORES // B)
        out[b] += res.results[c]["out"].astype(np.float64)
    return out.astype(np.float32)
